# revision 17
# baseline (speedup 1.0000x reference)
"""CaNetConv (GAT-style K-head gated graph attention) on 8 TRN2 NeuronCores.

v3: data-parallel over destination-node row slices (as baseline), restructured
phase 2 around the SWDGE descriptor-generation bottleneck (~13 ns per gathered
row on gpsimd):
  - ONE dma_gather stream per edge (1280B h_ext rows keyed by fc); the old
    per-edge ss gather (keyed by fr) is gone: s_src expansion now runs on the
    tensor engine as a 4-col matmul per tile, lhsT = host-shipped one-hot
    transpose ohT (edge->local-dest assignment is known at preprocess time),
    rhs = the window's contiguous ss rows.
  - Main accumulation: ONE 516-col fp8 matmul per 128-edge tile
    (lhsT = host-shipped unscaled one-hot, rhs = [h_k|1]*wq packed for all 4
    heads; the ones columns produce the denominators in the same matmul).
  - Per-window batched vector ops (z/lrelu/gs-scale) instead of per-tile
    [128x128] one-hot builds.
  - Phase-1 h_ext/ss_tab writes moved from gpsimd to the sync engine (HWDGE).
Phase 1 (tables via matmul from xT) and host edge sorting are the baseline's.
"""

import sys

for _p in ("/opt/trn_rl_repo", "/opt/pypackages",
           "/root/.axon_site/_ro/trn_rl_repo", "/root/.axon_site/_ro/pypackages"):
    if _p not in sys.path:
        sys.path.append(_p)

import os
import numpy as np
import ml_dtypes

N = 50000
E = 800000
D = 128
K = 4
P = 128
NCORES = 8
WPC = 49                 # windows (of 128 rows) per core
RPC = WPC * P            # 6272 rows per core
NPAD = NCORES * RPC      # 50176
BLK = 134                # per-head col block in h_ext row
ROW = 640                # h_ext row cols (bf16) -> 1280B
SS_OFF = K * BLK         # 536: s_src_{0..3} columns inside h_ext row
HSPLIT = 32768           # int16 split for fc gather
SSROW = 128              # ss_tab row cols (bf16) -> 256B
BF16 = ml_dtypes.bfloat16
FP8 = ml_dtypes.float8_e4m3
NT2 = (NPAD // P) // 2   # 196 phase-1 iterations (2 node tiles each)
SS_ITERS = (RPC + 255) // 256       # 25 phase-1 iters that write ss_tab
GCH = 8                  # max tiles (of 128 idxs) per dma_gather
ACC = 516                # fused accumulator cols: 4*[h(128)|den(1)]


def _wrap16(vals):
    """int16 index list -> [128, n/16] wrap layout (i -> [i%16 + 16c, i//16])."""
    n = len(vals)
    out = np.zeros((P, n // 16), dtype=np.int16)
    v = np.asarray(vals, dtype=np.int16).reshape(n // 16, 16)  # [col, lane]
    blk = v.T  # [16, n/16]
    for c in range(8):
        out[16 * c:16 * (c + 1), :] = blk
    return out


def _preprocess(x, adj, e, weights, a):
    row = adj[0].astype(np.int64)
    col = adj[1].astype(np.int64)
    keep = row != col
    fr = np.concatenate([row[keep], np.arange(N, dtype=np.int64)])
    fc = np.concatenate([col[keep], np.arange(N, dtype=np.int64)])

    order = np.argsort(fr, kind="stable")
    fr = fr[order]
    fc = fc[order]

    win = fr >> 7
    nwin_g = NPAD // P
    counts = np.bincount(win, minlength=nwin_g)
    starts = np.concatenate([[0], np.cumsum(counts)])

    # per (core, window) low/high edge lists in rotated numbering
    low_lists = {}
    high_lists = {}
    nL = np.zeros((NCORES, WPC), dtype=np.int64)
    nH = np.zeros((NCORES, WPC), dtype=np.int64)
    for c in range(NCORES):
        base = c * RPC
        for w in range(WPC):
            g = c * WPC + w
            s0, s1 = int(starts[g]), int(starts[g + 1])
            efc = (fc[s0:s1] - base) % NPAD
            elr = fr[s0:s1] - (g << 7)          # 0..127
            lo = efc < HSPLIT
            ol = np.argsort(efc[lo], kind="stable")
            oh = np.argsort(efc[~lo], kind="stable")
            low_lists[(c, w)] = (efc[lo][ol], elr[lo][ol])
            high_lists[(c, w)] = (efc[~lo][oh] - HSPLIT, elr[~lo][oh])
            nL[c, w] = int(lo.sum())
            nH[c, w] = len(efc) - int(lo.sum())

    tL = np.maximum(1, (nL.max(axis=0) + P - 1) // P)   # [WPC]
    tH = np.maximum(1, (nH.max(axis=0) + P - 1) // P)
    tpw = (tL + tH).astype(int)
    TT = int(tpw.sum())

    # ed16: gather idx only, [P, 16*TT]; ohu/ohT one-hot bits fp8, [P, 128*TT]
    ed16 = np.zeros((NCORES, P, 16 * TT), dtype=np.int16)
    ohu = np.zeros((NCORES, P, 128 * TT), dtype=BF16)
    oht = np.zeros((NCORES, P, 128 * TT), dtype=BF16)
    cum = np.concatenate([[0], np.cumsum(tpw)])
    for c in range(NCORES):
        for w in range(WPC):
            tl, th = int(tL[w]), int(tH[w])
            t = tl + th
            fcl, lrl = low_lists[(c, w)]
            fch, lrh = high_lists[(c, w)]
            gl = np.zeros(tl * P, dtype=np.int64)
            gh = np.zeros(th * P, dtype=np.int64)
            lr = np.full(t * P, -1, dtype=np.int64)
            gl[:len(fcl)] = fcl
            gh[:len(fch)] = fch
            lr[:len(lrl)] = lrl
            lr[tl * P:tl * P + len(lrh)] = lrh
            o16 = 16 * int(cum[w])
            ed16[c, :, o16:o16 + 8 * tl] = _wrap16(gl)
            ed16[c, :, o16 + 8 * tl:o16 + 8 * t] = _wrap16(gh)
            # one-hots per tile: ohu [e-part, m-col]; ohT [m-part, e-col]
            ob = 128 * int(cum[w])
            lrt = lr.reshape(t, P)  # [tile, e-slot]
            for j in range(t):
                m = lrt[j]
                valid = m >= 0
                u = np.zeros((P, P), dtype=BF16)
                u[np.arange(P)[valid], m[valid]] = 1.0
                ohu[c, :, ob + j * P:ob + (j + 1) * P] = u
                oht[c, :, ob + j * P:ob + (j + 1) * P] = u.T

    x_pad = np.zeros((NPAD, D), dtype=np.float32)
    x_pad[:N] = x
    e_pad = np.zeros((NPAD, K), dtype=np.float32)
    e_pad[:N] = e

    wext = np.zeros((D, ROW), dtype=np.float32)
    a1 = a[:, :D, 0]
    a2 = a[:, D:, 0]
    for k in range(K):
        wext[:, BLK * k:BLK * k + D] = weights[k]
        wext[:, BLK * k + 129] = weights[k] @ a2[k]
        wext[:, SS_OFF + k] = weights[k] @ a1[k]
    wext_bf = wext.astype(BF16)

    in_maps = []
    for c in range(NCORES):
        xr = np.roll(x_pad, -c * RPC, axis=0)
        xT_bf = np.ascontiguousarray(xr.T).astype(BF16)
        xe = np.zeros((WPC, P, D + K), dtype=np.float32)
        xe[:, :, :D] = x_pad[c * RPC:(c + 1) * RPC].reshape(WPC, P, D)
        xe[:, :, D:] = e_pad[c * RPC:(c + 1) * RPC].reshape(WPC, P, K)
        xepack = np.ascontiguousarray(
            xe.transpose(1, 0, 2).reshape(P, WPC * (D + K)))
        in_maps.append({
            "xT": xT_bf,
            "wext": wext_bf,
            "ed16": np.ascontiguousarray(ed16[c]),
            "ohu": np.ascontiguousarray(ohu[c]),
            "oht": np.ascontiguousarray(oht[c]),
            "xepack": xepack,
        })
    return in_maps, [int(v) for v in tL], [int(v) for v in tH], TT


def _build_graph(tL, tH, TT):
    WLIM = int(os.environ.get("KDBG_WLIM", WPC))
    from contextlib import ExitStack
    import concourse.bacc as bacc
    from concourse import bass, mybir
    from concourse.library_config import mlp

    f32 = mybir.dt.float32
    bf16 = mybir.dt.bfloat16
    fp8 = mybir.dt.float8e4
    i16 = mybir.dt.int16
    AF = mybir.ActivationFunctionType
    OP = mybir.AluOpType

    tpw = [a + b for a, b in zip(tL, tH)]
    TMAX = max(tpw)
    cum = [0]
    for t in tpw:
        cum.append(cum[-1] + t)

    def _chunks(nt):
        return [(a, min(a + GCH, nt)) for a in range(0, nt, GCH)]

    # gathers per window, cumulative per window-parity
    gpw = [len(_chunks(tL[w])) + len(_chunks(tH[w])) for w in range(WLIM)]
    gcum_par = [[0] * (WLIM + 1), [0] * (WLIM + 1)]
    for w in range(WLIM):
        for p_ in (0, 1):
            gcum_par[p_][w + 1] = gcum_par[p_][w] + (gpw[w] if w % 2 == p_ else 0)

    nc = bacc.Bacc("TRN2", num_swdge_queues=2)
    xT = nc.declare_dram_parameter("xT", [P, NPAD], bf16, isOutput=False)
    wext = nc.declare_dram_parameter("wext", [P, ROW], bf16, isOutput=False)
    ed16 = nc.declare_dram_parameter("ed16", [P, 16 * TT], i16, isOutput=False)
    ohu_d = nc.declare_dram_parameter("ohu", [P, 128 * TT], bf16, isOutput=False)
    oht_d = nc.declare_dram_parameter("oht", [P, 128 * TT], bf16, isOutput=False)
    xepack = nc.declare_dram_parameter("xepack", [P, WPC * (D + K)], f32,
                                       isOutput=False)
    out_ext = nc.declare_dram_parameter("out", [RPC, D], f32, isOutput=True)
    h_ext = nc.dram_tensor("h_ext", [NPAD, ROW], bf16)
    ss_tab = nc.dram_tensor("ss_tab", [NPAD, SSROW], bf16)

    # phase-1 write count (h_ext per iter + ss per early iter), on sync engine
    NWR = NT2 + SS_ITERS

    with ExitStack() as ctx:
        def sb(nm, shape, dt_):
            return ctx.enter_context(nc.sbuf_tensor(nm, shape, dt_))

        def sem(name):
            return ctx.enter_context(nc.semaphore(name))

        wext_sb = sb("wext_sb", [P, ROW], bf16)
        xt2 = sb("xt2", [P, 2 * 2 * P], bf16)
        hb2 = sb("hb2", [P, 2 * 2 * ROW], bf16)
        ed2 = sb("ed2", [P, 2 * 16 * TMAX], i16)
        oh2u = sb("oh2u", [P, 2 * 128 * TMAX], bf16)
        oh2t = sb("oh2t", [P, 2 * 128 * TMAX], bf16)
        xe2 = sb("xe2", [P, 2 * (D + K)], f32)
        ssw2 = sb("ssw2", [P, 2 * K], bf16)
        g2 = sb("g2", [P, 2 * TMAX * ROW], bf16)
        gs2 = sb("gs2", [P, 2 * TMAX * ACC], bf16)
        z2 = sb("z2", [P, 2 * K * TMAX], bf16)
        u2 = sb("u2", [P, 2 * K * TMAX], bf16)
        wq2 = sb("wq2", [P, 2 * K * TMAX], bf16)
        dn_sb = sb("dn_sb", [P, K], f32)
        rec_sb = sb("rec_sb", [P, K], f32)
        sc_sb = sb("sc_sb", [P, K], f32)
        ot2 = sb("ot2", [P, 2 * D], f32)
        otx = sb("otx", [P, 2 * D], f32)
        ps = ctx.enter_context(nc.psum_tensor("ps", [P, 4096], f32))

        s_wx = sem("s_wx")
        s_xt = [sem("s_xt0"), sem("s_xt1")]
        s_mm1 = sem("s_mm1")
        s_ev = sem("s_ev")
        s_evd = sem("s_evd")
        s_hw = sem("s_hw")
        s_ed = [sem("s_ed0"), sem("s_ed1")]
        s_oh = [sem("s_oh0"), sem("s_oh1")]
        s_xe = [sem("s_xe0"), sem("s_xe1")]
        s_ss = [sem("s_ss0"), sem("s_ss1")]
        s_g = [sem("s_g0"), sem("s_g1")]
        s_s2e = sem("s_s2e")
        s_z = sem("s_z")
        s_u = sem("s_u")
        s_wq = sem("s_wq")
        s_gs = sem("s_gs")
        s_ones = sem("s_ones")
        s_pe = sem("s_pe")
        s_ep = sem("s_ep")
        s_ow = [sem("s_ow0"), sem("s_ow1")]
        s_init = sem("s_init")

        def xt_t(i, s):
            b = (i % 2) * 2 * P
            return xt2[:, b + s * P: b + (s + 1) * P]

        def hb_blk(i, s):
            b = (i % 2) * 2 * ROW
            return hb2[:, b + s * ROW: b + (s + 1) * ROW]

        def hb_full(i):
            b = (i % 2) * 2 * ROW
            return hb2[:, b: b + 2 * ROW]

        def ps1(i, s):
            b = (i % 2) * 2048 + s * 1024
            return ps[:, b: b + 1024]

        # phase-2 psum: acc A split in two bank-aligned halves (258 cols each,
        # heads 0,1 then heads 2,3) at 0/512 and 2048/2560; s2e B at 1024 / 3072
        def accA(w, half):
            b = (w % 2) * 2048 + half * 512
            return ps[:, b: b + 258]

        def accN(w, k):
            b = (w % 2) * 2048 + (k // 2) * 512 + 129 * (k % 2)
            return ps[:, b: b + 128]

        def accD(w, half):
            b = (w % 2) * 2048 + half * 512 + 128
            return ps[:, b: b + 258].rearrange("p (k c) -> p k c", k=2)[:, :, 0]

        def accB(w):
            b = 1024 + (w % 2) * 2048
            return ps[:, b: b + K * TMAX]

        def ed_sl(w):
            b = (w % 2) * 16 * TMAX
            return ed2[:, b: b + 16 * tpw[w]]

        def ohu_sl(w):
            b = (w % 2) * 128 * TMAX
            return oh2u[:, b: b + 128 * tpw[w]]

        def oht_sl(w):
            b = (w % 2) * 128 * TMAX
            return oh2t[:, b: b + 128 * tpw[w]]

        def xe_sl(w):
            b = (w % 2) * (D + K)
            return xe2[:, b: b + D + K]

        def g_sl(w):
            b = (w % 2) * TMAX * ROW
            return g2[:, b: b + tpw[w] * ROW]

        def gs_sl(w):
            b = (w % 2) * TMAX * ACC
            return gs2[:, b: b + tpw[w] * ACC]

        def z_sl(w):
            b = (w % 2) * K * TMAX
            return z2[:, b: b + K * tpw[w]]

        def u_sl(w):
            b = (w % 2) * K * TMAX
            return u2[:, b: b + K * tpw[w]]

        def wq_sl(w):
            b = (w % 2) * K * TMAX
            return wq2[:, b: b + K * tpw[w]]

        def ot_sl(w):
            b = (w % 2) * D
            return ot2[:, b: b + D]

        with nc.Block() as block:

            @block.sync
            def _(sp):
                def p1_write_sp(j):
                    sp.wait_ge(s_ev, j + 1)
                    sp.wait_ge(s_evd, j + 1)
                    sp.wait_ge(s_ones, j + 1)
                    dstv = h_ext[j * 2 * P:(j + 1) * 2 * P, :].rearrange(
                        "(s p) c -> p s c", p=P)
                    srcv = hb_full(j).rearrange("p (s c) -> p s c", s=2)
                    sp.dma_start(out=dstv, in_=srcv).then_inc(s_hw, 16)

                sp.dma_start(out=wext_sb[:], in_=wext[:]).then_inc(s_wx, 16)
                for i in range(NT2):
                    if i >= 2:
                        sp.wait_ge(s_mm1, 4 * (i - 1))
                    sp.dma_start(
                        out=xt2[:, (i % 2) * 2 * P:(i % 2 + 1) * 2 * P],
                        in_=xT[:, i * 2 * P:(i + 1) * 2 * P],
                    ).then_inc(s_xt[i % 2], 16)
                    if i >= 1 and (i - 1) % 2 == 0:
                        p1_write_sp(i - 1)
                if (NT2 - 1) % 2 == 0:
                    p1_write_sp(NT2 - 1)
                # phase-2 per-window loads
                for w in range(WLIM):
                    if w >= 2:
                        sp.wait_ge(s_pe, w - 1)   # oh slot free
                        sp.wait_ge(s_ep, w - 1)
                        sp.wait_ge(s_g[w % 2], 16 * gcum_par[w % 2][w - 1])
                    sp.dma_start(
                        out=ed_sl(w),
                        in_=ed16[:, 16 * cum[w]: 16 * cum[w] + 16 * tpw[w]],
                    ).then_inc(s_ed[w % 2], 16)
                    sp.dma_start(
                        out=ohu_sl(w),
                        in_=ohu_d[:, 128 * cum[w]: 128 * (cum[w] + tpw[w])],
                    ).then_inc(s_oh[w % 2], 16)
                    sp.dma_start(
                        out=oht_sl(w),
                        in_=oht_d[:, 128 * cum[w]: 128 * (cum[w] + tpw[w])],
                    ).then_inc(s_oh[w % 2], 16)
                    sp.dma_start(
                        out=ssw2[:, (w % 2) * K:(w % 2 + 1) * K],
                        in_=ss_tab[w * P:(w + 1) * P, 0:K]
                        .rearrange("(s p) c -> p (s c)", p=P),
                    ).then_inc(s_ss[w % 2], 16)
                    sp.dma_start(
                        out=xe_sl(w),
                        in_=xepack[:, w * (D + K):(w + 1) * (D + K)],
                    ).then_inc(s_xe[w % 2], 16)

            @block.tensor
            def _(t):
                t.wait_ge(s_wx, 16)
                for i in range(NT2):
                    t.wait_ge(s_xt[i % 2], 16 * (i // 2 + 1))
                    if i >= 2:
                        t.wait_ge(s_ev, i - 1)
                        t.wait_ge(s_evd, i - 1)
                    for s in (0, 1):
                        pp = ps1(i, s)
                        nc.tensor.matmul(
                            out=pp[:, 0:320], lhsT=xt_t(i, s),
                            rhs=wext_sb[:, 0:320], start=True, stop=True,
                        ).then_inc(s_mm1, 1)
                        nc.tensor.matmul(
                            out=pp[:, 512:832], lhsT=xt_t(i, s),
                            rhs=wext_sb[:, 320:640], start=True, stop=True,
                        ).then_inc(s_mm1, 1)
                # phase 2: per window: s2e mms for w, then main mms for w-1
                t.wait_ge(s_ev, NT2)
                t.wait_ge(s_evd, NT2)
                for w in range(WLIM + 1):
                    if w < WLIM:
                        tw = tpw[w]
                        t.wait_ge(s_oh[w % 2], 32 * (w // 2 + 1))
                        t.wait_ge(s_ss[w % 2], 16 * (w // 2 + 1))
                        if w >= 2:
                            t.wait_ge(s_z, w - 1)   # psum B slot free
                        for j in range(tw):
                            ins = nc.tensor.matmul(
                                out=accB(w)[:, j * K:(j + 1) * K],
                                lhsT=oht_sl(w)[:, j * P:(j + 1) * P],
                                rhs=ssw2[:, (w % 2) * K:(w % 2 + 1) * K],
                                start=True, stop=True,
                            )
                        ins.then_inc(s_s2e, 1)
                    if w >= 1:
                        v = w - 1
                        tv = tpw[v]
                        t.wait_ge(s_gs, v + 1)
                        if v >= 2:
                            t.wait_ge(s_ep, v - 1)  # psum A slot free
                        for j in range(tv):
                            for hf in (0, 1):
                                ins = nc.tensor.matmul(
                                    out=accA(v, hf),
                                    lhsT=ohu_sl(v)[:, j * P:(j + 1) * P],
                                    rhs=gs_sl(v)[:, j * ACC + 258 * hf:
                                                 j * ACC + 258 * (hf + 1)],
                                    start=(j == 0), stop=(j == tv - 1),
                                )
                        ins.then_inc(s_pe, 1)

            @block.scalar
            def _(sc):
                def p1_write_sc(j):
                    sc.wait_ge(s_evd, j + 1)
                    sc.wait_ge(s_ones, j + 1)
                    if j < SS_ITERS:
                        ssrc = hb_full(j).rearrange(
                            "p (s c) -> p s c", s=2)[:, :, SS_OFF:SS_OFF + K]
                        sdst = ss_tab[j * 2 * P:(j + 1) * 2 * P, 0:K] \
                            .rearrange("(s p) c -> p s c", p=P)
                        sc.dma_start(out=sdst, in_=ssrc).then_inc(s_hw, 16)
                    if j % 2 == 1:
                        dstv = h_ext[j * 2 * P:(j + 1) * 2 * P, :].rearrange(
                            "(s p) c -> p s c", p=P)
                        srcv = hb_full(j).rearrange("p (s c) -> p s c", s=2)
                        sc.dma_start(out=dstv, in_=srcv).then_inc(s_hw, 16)

                for i in range(NT2):
                    sc.wait_ge(s_mm1, 4 * i + 2)
                    if i >= 2:
                        sc.wait_ge(s_hw, 16 * _wr_thru(i - 2))
                    src = ps1(i, 0).rearrange("p (b c) -> p b c", b=2)[:, :, 0:320]
                    dst = hb_blk(i, 0).rearrange("p (b c) -> p b c", b=2)
                    sc.activation(out=dst, in_=src, func=AF.Copy).then_inc(s_ev, 1)
                    if i >= 1:
                        p1_write_sc(i - 1)
                p1_write_sc(NT2 - 1)
                for w in range(WLIM):
                    # z0 = copy of s2e psum (f32 -> bf16)
                    sc.wait_ge(s_s2e, w + 1)
                    if w >= 2:
                        sc.wait_ge(s_u, w - 1)   # z slot free
                    sc.activation(out=z_sl(w),
                                  in_=accB(w)[:, 0:K * tpw[w]],
                                  func=AF.Copy).then_inc(s_z, 1)
                    # wq = exp(u)
                    sc.wait_ge(s_u, w + 1)
                    sc.activation(out=wq_sl(w), in_=u_sl(w),
                                  func=AF.Exp).then_inc(s_wq, 1)
                    if w >= 1:
                        sc.wait_ge(s_ep, w)
                        sc.dma_start(
                            out=out_ext[(w - 1) * P: w * P, :],
                            in_=ot_sl(w - 1),
                        ).then_inc(s_ow[(w - 1) % 2], 16)
                if WLIM > 0:
                    sc.wait_ge(s_ep, WLIM)
                    sc.dma_start(
                        out=out_ext[(WLIM - 1) * P: WLIM * P, :],
                        in_=ot_sl(WLIM - 1),
                    ).then_inc(s_ow[(WLIM - 1) % 2], 16)

            @block.vector
            def _(v):
                for i in range(NT2):
                    v.wait_ge(s_mm1, 4 * i + 4)
                    if i >= 2:
                        v.wait_ge(s_hw, 16 * _wr_thru(i - 2))
                    src = ps1(i, 1).rearrange("p (b c) -> p b c", b=2)[:, :, 0:320]
                    dst = hb_blk(i, 1).rearrange("p (b c) -> p b c", b=2)
                    v.tensor_copy(out=dst, in_=src).then_inc(s_evd, 1)

                def epilogue(u_):
                    v.wait_ge(s_pe, u_ + 1)
                    v.wait_ge(s_xe[u_ % 2], 16 * (u_ // 2 + 1))
                    v.tensor_scalar_add(dn_sb[:, 0:2], accD(u_, 0), 1e-8)
                    v.tensor_scalar_add(dn_sb[:, 2:4], accD(u_, 1), 1e-8)
                    v.drain()
                    v.reciprocal(rec_sb[:], dn_sb[:])
                    v.drain()
                    v.tensor_tensor(out=sc_sb[:], in0=rec_sb[:],
                                    in1=xe_sl(u_)[:, D:D + K], op=OP.mult)
                    v.drain()
                    if u_ >= 2:
                        v.wait_ge(s_ow[u_ % 2], 16 * (u_ // 2))
                    xb = (u_ % 2) * D
                    bufs = [xe_sl(u_)[:, 0:D],
                            otx[:, xb:xb + D],
                            ot2[:, xb:xb + D],
                            otx[:, xb:xb + D],
                            ot2[:, xb:xb + D]]
                    for k in range(K):
                        ins2 = v.scalar_tensor_tensor(
                            out=bufs[k + 1], in0=accN(u_, k),
                            scalar=sc_sb[:, k:k + 1], in1=bufs[k],
                            op0=OP.mult, op1=OP.add)
                    ins2.then_inc(s_ep, 1)

                for w in range(WLIM):
                    tw = tpw[w]
                    # u = lrelu(z0 + sd)
                    v.wait_ge(s_z, w + 1)
                    v.wait_ge(s_g[w % 2], 16 * gcum_par[w % 2][w + 1])
                    sd_ap = g_sl(w).rearrange(
                        "p (j c) -> p j c", c=ROW)[:, :, 0:K * BLK].rearrange(
                        "p j (k c) -> p j k c", c=BLK)[:, :, :, 129]
                    v.tensor_tensor(
                        out=z_sl(w).rearrange("p (j k) -> p j k", k=K),
                        in0=z_sl(w).rearrange("p (j k) -> p j k", k=K),
                        in1=sd_ap, op=OP.add)
                    v.scalar_tensor_tensor(
                        out=u_sl(w), in0=z_sl(w), scalar=0.01, in1=z_sl(w),
                        op0=OP.mult, op1=OP.max).then_inc(s_u, 1)
                    # gs: per head, [h_k|1]*wq -> fp8, all tiles batched
                    v.wait_ge(s_wq, w + 1)
                    if w >= 2:
                        v.wait_ge(s_pe, w - 1)   # gs slot free (main mms done)
                    for k in range(K):
                        ins = v.tensor_tensor(
                            out=gs_sl(w).rearrange(
                                "p (j c) -> p j c", c=ACC)[:, :, 129 * k:129 * (k + 1)],
                            in0=g_sl(w).rearrange(
                                "p (j c) -> p j c", c=ROW)[:, :, BLK * k:BLK * k + 129],
                            in1=wq_sl(w).rearrange(
                                "p (j k) -> p j k", k=K)[:, :, k:k + 1]
                            .to_broadcast([P, tw, 129]),
                            op=OP.mult)
                    ins.then_inc(s_gs, 1)
                    if w >= 1:
                        epilogue(w - 1)
                if WLIM > 0:
                    epilogue(WLIM - 1)

            @block.gpsimd
            def _(g):
                g.load_library(mlp)
                # phase 1: ones columns (head block col 128 -> denominator 1s)
                for i in range(NT2):
                    g.wait_ge(s_ev, i + 1)
                    g.wait_ge(s_evd, i + 1)
                    for s in (0, 1):
                        ones_ap = hb_blk(i, s)[:, 0:K * BLK].rearrange(
                            "p (k c) -> p k c", k=K)[:, :, 128:129]
                        ins = g.memset(ones_ap, 1.0)
                    ins.then_inc(s_ones, 1)
                g.wait_ge(s_hw, 16 * NWR)
                g.wait_ge(s_ed[0], 16)
                # warm-up gather (first gather after Q7 load can misread idxs)
                g.dma_gather(
                    gs2[:, 0:128].rearrange(
                        "p (t c) -> p t c", c=128),
                    h_ext[0:HSPLIT, 0:128], ed2[:, 0:8], P, P, 128,
                    elem_step=ROW, queue_num=0,
                ).then_inc(s_init, 16)
                g.wait_ge(s_init, 16)
                qn = 0
                for w in range(WLIM):
                    g.wait_ge(s_ed[w % 2], 16 * (w // 2 + 1))
                    if w >= 2:
                        g.wait_ge(s_gs, w - 1)   # g2 slot free
                    tl, th = tL[w], tH[w]
                    e0 = 16 * ((w % 2) * TMAX)
                    eb = ed2[:, e0: e0 + 16 * (tl + th)]
                    for (a, b) in _chunks(tl):
                        n = (b - a) * P
                        g.dma_gather(
                            g_sl(w)[:, a * ROW:b * ROW].rearrange(
                                "p (t c) -> p t c", c=ROW),
                            h_ext[0:HSPLIT, :], eb[:, 8 * a:8 * b],
                            n, n, ROW, queue_num=qn,
                        ).then_inc(s_g[w % 2], 16)
                        qn ^= 1
                    for (a, b) in _chunks(th):
                        n = (b - a) * P
                        g.dma_gather(
                            g_sl(w)[:, (tl + a) * ROW:(tl + b) * ROW].rearrange(
                                "p (t c) -> p t c", c=ROW),
                            h_ext[HSPLIT:NPAD, :],
                            eb[:, 8 * (tl + a):8 * (tl + b)],
                            n, n, ROW, queue_num=qn,
                        ).then_inc(s_g[w % 2], 16)
                        qn ^= 1

    nc.compile()
    return nc


def _wr_thru(i):
    """sync-engine table writes issued through phase-1 iter i (1 or 2 per iter)."""
    return (i + 1) + min(i + 1, SS_ITERS)


def kernel(x, adj, e, weights, a):
    from concourse.bass_utils import run_bass_kernel_spmd

    x = np.asarray(x, dtype=np.float32)
    adj = np.asarray(adj)
    e = np.asarray(e, dtype=np.float32)
    weights = np.asarray(weights, dtype=np.float32)
    a = np.asarray(a, dtype=np.float32)

    in_maps, tL, tH, TT = _preprocess(x, adj, e, weights, a)
    nc = _build_graph(tL, tH, TT)
    res = run_bass_kernel_spmd(nc, in_maps, core_ids=list(range(NCORES)))
    outs = [res.results[c]["out"] for c in range(NCORES)]
    full = np.concatenate(outs, axis=0)
    return full[:N].astype(np.float32)
